# revision 29
# baseline (speedup 1.0000x reference)
"""Trainium2 Bass kernel for the non-local (self-attention over spatial
positions) block.

Per batch b (8 batches -> one per NeuronCore):
    xf    = x[b]                       [C=128, N=4096]
    theta = w_theta @ xf               [64, N]
    phi   = w_phi   @ xf               [64, N]
    g     = w_g     @ xf               [64, N]
    attn  = softmax(theta^T phi)       [N, N]   (softmax over keys m)
    y     = g @ attn^T                 [64, N]
    out   = w_last @ y + xf            [128, N]

Design (per core), final (~157 us vs 346 us baseline):
 - scoresT[m, q] orientation (phi tiles stationary) so exp(scoresT)
   feeds the y matmul directly as the moving operand.
 - fp16 theta/phi: the PE moving-operand path is ~256 B/cycle, so the
   two concurrent 64-row score matmuls (m-tile pair in disjoint PE row
   groups) stream 2 cols/cycle -- 2x over f32r at any clock, and
   fp16's 10-bit mantissa keeps logit error ~10x below bf16's.
   (bf16 operands everywhere failed the 2e-2 gate; f32 dram bits fed
   straight into f32r matmuls compute garbage on hardware.)
 - Keeping total PE load low and never stalling it keeps the tensor
   clock at 2.4 GHz instead of the 1.2 GHz it drops to under stalls.
 - exp (16.8M elements, the #2 load) split 20/12 between ACT (real
   Exp) and DVE (Schraudolph bit-trick exp: one tensor_scalar affine
   with int16 output bitcast as bf16), interleaved evenly so neither
   engine builds a backlog. y/w_last matmuls are uniform bf16 (range
   needs bf16: unnormalized y and row sums reach e^44).
 - Row sums via a ones column in gt. The per-chunk epilogue is staged
   across the next chunk's pair stream; the rowsum reciprocal runs as
   [128, 4] via an SBUF->SBUF DMA scatter/gather (a [1, 512] DVE
   reciprocal costs 3.3 us and starves the exp stream).
 - Input DMA / projections / first q-chunk fused so the PE starts
   while the input streams in; weights DMA'd before the bulk input.
"""

import sys

import numpy as np

for _p in ("/opt/trn_rl_repo",):
    if _p not in sys.path:
        sys.path.insert(0, _p)

import concourse.bass as bass
from concourse import bacc
import concourse.mybir as mybir
import concourse.tile as tile
from concourse.alu_op_type import AluOpType
from concourse.bass_utils import run_bass_kernel_spmd

F32 = mybir.dt.float32
F16 = mybir.dt.float16
BF16 = mybir.dt.bfloat16
I16 = mybir.dt.int16

P = 128     # channels C / partition dim
CB = 64     # bottleneck channels
NQ = 4096   # spatial positions (64*64)
NMT = 32    # m (key) tiles of 128
NPAIR = 16  # m-tile pairs per q chunk
NQC = 8     # q chunks of 512

# Schraudolph exp in bf16: exp(s) ~= bitcast<bf16>(int16(A*s + B))
EXP_A = float(2**7 / np.log(2.0))
EXP_B = float(127.0 * 2**7 - 6.0)
# exp engine split: DVE takes the second tile of pairs not divisible
# by 4 (12 of 32 tiles, evenly interleaved); ACT takes the other 20.
# 16/16 measured slower (DVE becomes co-pacer with its epilogue work).
def _exp_on_dve(mi):
    return (mi % 2 == 1) and ((mi // 2) % 4 != 0)

_NC_CACHE = {}


def _build():
    nc = bacc.Bacc()
    x_in = nc.declare_dram_parameter("xb", [P, NQ], F32, isOutput=False)
    wqk_in = nc.declare_dram_parameter("wqk", [P, P], F32, isOutput=False)
    wg_in = nc.declare_dram_parameter("wgT", [P, CB], F32, isOutput=False)
    wl_in = nc.declare_dram_parameter("wl", [CB, P], F32, isOutput=False)
    out_d = nc.declare_dram_parameter("out", [P, NQ], F32, isOutput=True)

    with tile.TileContext(nc) as tc:
        with (
            tc.tile_pool(name="const", bufs=1) as const,
            tc.tile_pool(name="big", bufs=1) as big,
            tc.tile_pool(name="work", bufs=2) as work,
            tc.tile_pool(name="probs", bufs=9) as probs,
            tc.tile_pool(name="spool", bufs=7, space="PSUM") as spool,
            tc.tile_pool(name="ypool", bufs=1, space="PSUM") as ypool,
        ):
            # ---- small weights first (needed before any compute) ----
            wqk_f = const.tile([P, P], F32)
            wg_f = const.tile([P, CB], F32)
            wl_f = const.tile([CB, P], F32)
            nc.sync.dma_start(out=wqk_f, in_=wqk_in[:, :])
            nc.sync.dma_start(out=wg_f, in_=wg_in[:, :])
            nc.sync.dma_start(out=wl_f, in_=wl_in[:, :])
            wqk = const.tile([P, P], F16)
            wg = const.tile([P, CB], F16)
            wl = const.tile([CB, P], BF16)
            nc.vector.tensor_copy(wqk, wqk_f)
            nc.vector.tensor_copy(wg, wg_f)
            nc.vector.tensor_copy(wl, wl_f)

            xb = big.tile([P, NQ], F32)
            xb16 = big.tile([P, NQ], F16)
            # theta/phi duplicated into both partition halves so score
            # matmuls for two m-tiles run concurrently in PE row groups
            theta = big.tile([P, NQ], F16)
            phi = big.tile([P, NQ], F16)
            # gT in 65-col slots; col 64 = ones for the row-sum trick
            gt = big.tile([P, NMT * (CB + 1)], BF16)
            nc.vector.memset(gt, 1.0)
            gt3 = gt.rearrange("p (m c) -> p m c", c=CB + 1)

            # ---------------- pipelined helpers ----------------
            qof = [qc * 512 for qc in range(NQC)]

            def score_pair(qc, pi):
                """Two concurrent 64-row score matmuls for m-tiles
                2*pi (rows 0:64) and 2*pi+1 (rows 64:128)."""
                q = qof[qc]
                sa = spool.tile([P, 512], F32, tag="s")
                nc.tensor.matmul(
                    sa, phi[0:CB, (2 * pi) * 128:(2 * pi + 1) * 128],
                    theta[0:CB, q:q + 512], start=True, stop=True,
                )
                sb = spool.tile([P, 512], F32, tag="s")
                nc.tensor.matmul(
                    sb, phi[CB:P, (2 * pi + 1) * 128:(2 * pi + 2) * 128],
                    theta[CB:P, q:q + 512], start=True, stop=True,
                )
                return sa, sb

            def exp_tile(mi, sp, split=False):
                pb = probs.tile([P, 512], BF16, tag="pb")
                if split:
                    # tail of the last chunk: halve drain latency by
                    # computing one half on each engine concurrently
                    nc.scalar.activation(
                        pb[:, 0:256], sp[:, 0:256],
                        mybir.ActivationFunctionType.Exp,
                    )
                    nc.vector.tensor_scalar(
                        pb.bitcast(I16)[:, 256:512], sp[:, 256:512],
                        EXP_A, EXP_B, AluOpType.mult, AluOpType.add,
                    )
                elif _exp_on_dve(mi):
                    nc.vector.tensor_scalar(
                        pb.bitcast(I16), sp, EXP_A, EXP_B,
                        AluOpType.mult, AluOpType.add,
                    )
                elif (mi % 2 == 0) and ((mi // 2) % 4 == 0):
                    # ACT owns both tiles of this pair; splitting one tile
                    # keeps ACT's per-pair load at 1030ns instead of 1374
                    # (the 870ns pair cadence otherwise builds a backlog)
                    nc.scalar.activation(
                        pb[:, 0:256], sp[:, 0:256],
                        mybir.ActivationFunctionType.Exp,
                    )
                    nc.vector.tensor_scalar(
                        pb.bitcast(I16)[:, 256:512], sp[:, 256:512],
                        EXP_A, EXP_B, AluOpType.mult, AluOpType.add,
                    )
                else:
                    nc.scalar.activation(
                        pb, sp, mybir.ActivationFunctionType.Exp
                    )
                return pb

            def y_mm(yps, mi, pb):
                nc.tensor.matmul(
                    yps,
                    gt[:, mi * (CB + 1):(mi + 1) * (CB + 1)],
                    pb,
                    start=(mi == 0), stop=(mi == NMT - 1),
                )

            # per-chunk epilogue, staged across the next chunk's pair
            # stream so no single engine stalls the PE. The rowsum
            # reciprocal is done as [128, 4] via an SBUF->SBUF DMA
            # scatter/gather: a [1, 512] DVE reciprocal costs 3.3us
            # (free-size bound) and starves the DVE exp stream, while
            # [128, 4] costs ~0.25us.
            #   chunk end: yu + rowsum copies (DVE, frees yps)
            #   pi==2: DMA scatter rowsum -> [128, 4]
            #   pi==3: reciprocal [128, 4] (DVE)
            #   pi==4: DMA gather -> [1, 512]
            #   pi==5: partition broadcast (gpsimd)
            #   pi==6: w_last matmul (PE)
            #   pi==7: normalize multiply (DVE, straight out of PSUM)
            #   pi==8: residual add (DVE)
            #   pi==9: output DMA
            def epi_stage(st, stage):
                qc = st["qc"]
                if stage == 2:
                    st["rsq"] = work.tile([P, 4], F32, tag="rsq", name="rsq")
                    nc.sync.dma_start(out=st["rsq"], in_=st["rs"])
                elif stage == 3:
                    st["rqi"] = work.tile([P, 4], F32, tag="rqi", name="rqi")
                    nc.vector.reciprocal(st["rqi"], st["rsq"])
                elif stage == 4:
                    st["rinv"] = work.tile([1, 512], F32, tag="rinv", name="rinv")
                    nc.sync.dma_start(out=st["rinv"], in_=st["rqi"])
                elif stage == 5:
                    st["rb"] = work.tile([P, 512], F32, tag="rb", name="rb")
                    nc.gpsimd.partition_broadcast(st["rb"], st["rinv"])
                elif stage == 6:
                    st["op"] = spool.tile([P, 512], F32, tag="s", name="op")
                    nc.tensor.matmul(st["op"], wl, st["yu"],
                                     start=True, stop=True)
                elif stage == 7:
                    st["ob"] = work.tile([P, 512], F32, tag="ob", name="ob")
                    nc.vector.tensor_mul(st["ob"], st["op"], st["rb"])
                elif stage == 8:
                    st["ob2"] = work.tile([P, 512], F32, tag="ob2", name="ob2")
                    nc.vector.tensor_add(
                        st["ob2"], st["ob"], xb[:, qof[qc]:qof[qc] + 512]
                    )
                elif stage == 9:
                    nc.sync.dma_start(
                        out=out_d[:, qof[qc]:qof[qc] + 512], in_=st["ob2"]
                    )

            EPI_STAGES = range(2, 10)

            def epi_begin(qc, yps):
                # rs first: it heads the reciprocal DMA chain (longest
                # pole of the epilogue); nothing needs yu until pi==6
                rs = work.tile([1, 512], F32, tag="rs")
                nc.vector.tensor_copy(rs, yps[CB:CB + 1, :])
                yu = work.tile([CB, 512], BF16, tag="yu")
                nc.vector.tensor_copy(yu, yps[0:CB, :])  # frees yps
                return {"qc": qc, "yps": yps, "yu": yu, "rs": rs}

            def pair_tiles(qc, yps, pi, look=2):
                """Scores+exp for pair pi, y matmuls for pair pi-look."""
                sa, sb = score_pair(qc, pi)
                split = (qc == NQC - 1 and pi >= NPAIR - 3)
                pbq[2 * pi] = exp_tile(2 * pi, sa, split)
                pbq[2 * pi + 1] = exp_tile(2 * pi + 1, sb, split)
                pj = pi - look
                if pj >= 0:
                    for mi in (2 * pj, 2 * pj + 1):
                        y_mm(yps, mi, pbq.pop(mi))

            def drain_y(yps, look=2):
                for pj in range(NPAIR - look, NPAIR):
                    for mi in (2 * pj, 2 * pj + 1):
                        y_mm(yps, mi, pbq.pop(mi))

            # ---------------- init fused with q-chunk 0 ----------------
            # Per 512-col xb chunk j: DMA, theta/phi projection, 4 gT
            # projections; from j>=1 also run q-chunk-0 score/exp/y for
            # the m-tile pairs whose phi/gt landed in iteration j-1.
            yps0 = ypool.tile([CB + 1, 512], F32, tag="y")
            pbq = {}  # mi -> pb tile awaiting its y matmul

            for j in range(8):
                cs = slice(j * 512, (j + 1) * 512)
                # split DMA issue across both hwdge queues (SP + ACT) so
                # the per-dma DGE setup doesn't serialize the input stream
                if j % 2 == 0:
                    nc.sync.dma_start(out=xb[:, cs], in_=x_in[:, cs])
                else:
                    nc.scalar.dma_start(out=xb[:, cs], in_=x_in[:, cs])
                nc.scalar.copy(xb16[:, cs], xb[:, cs])
                ps = spool.tile([P, 512], F32, tag="s")
                nc.tensor.matmul(ps, wqk, xb16[:, cs], start=True, stop=True)
                # theta lower half is partition-aligned -> ACT engine;
                # the shifted copies (DVE only) fill the other halves
                nc.scalar.copy(theta[0:CB, cs], ps[0:CB, :])
                nc.vector.tensor_copy(phi[0:CB, cs], ps[CB:P, :])
                nc.vector.tensor_copy(phi[CB:P, cs], ps[CB:P, :])
                if j == 0:
                    nc.vector.tensor_copy(theta[CB:P, cs], ps[0:CB, :])
                gp = spool.tile([P, 512], F32, tag="s")
                gp3 = gp.rearrange("p (m c) -> p m c", c=CB)
                for k in range(4):
                    mi = 4 * j + k
                    nc.tensor.matmul(
                        gp3[:, k, :], xb16[:, mi * 128:(mi + 1) * 128], wg,
                        start=True, stop=True,
                    )
                nc.scalar.copy(gt3[:, 4 * j:4 * j + 4, 0:CB], gp3[:, 0:4, :])
                if j >= 1:
                    for pi in (2 * (j - 1), 2 * (j - 1) + 1):
                        pair_tiles(0, yps0, pi)
            for pi in (14, 15):
                pair_tiles(0, yps0, pi)
            drain_y(yps0)
            # upper-theta for chunk 1 (needed at its start)
            c1 = slice(512, 1024)
            nc.vector.tensor_copy(theta[CB:P, c1], theta[0:CB, c1])
            pending = epi_begin(0, yps0)

            # ---------------- q-chunks 1..7 ----------------
            for qc in range(1, NQC):
                yps = ypool.tile([CB + 1, 512], F32, tag="y")
                for pi in range(NPAIR):
                    pair_tiles(qc, yps, pi)
                    if pending is not None and 2 <= pi <= 9:
                        epi_stage(pending, pi)
                    if pi == 8 and qc < NQC - 1:
                        # upper-theta for the next chunk, off-peak on DVE
                        cn = slice(qof[qc + 1], qof[qc + 1] + 512)
                        nc.vector.tensor_copy(theta[CB:P, cn], theta[0:CB, cn])
                drain_y(yps)
                pending = epi_begin(qc, yps)

            # final epilogue: w_last matmul (stage 6) early -- it only
            # needs yu, and runs on the PE concurrently with the
            # reciprocal DMA round-trip
            for stage in (2, 6, 3, 4, 5, 7, 8, 9):
                epi_stage(pending, stage)

    nc.finalize()
    return nc


def kernel(x, w_theta, w_phi, w_g, w_last):
    B, C, H, W = x.shape
    N = H * W
    xf = np.ascontiguousarray(x.reshape(B, C, N), dtype=np.float32)
    wqk = np.ascontiguousarray(
        np.concatenate([w_theta.T, w_phi.T], axis=1), dtype=np.float32
    )
    wgT = np.ascontiguousarray(w_g.T, dtype=np.float32)
    wl = np.ascontiguousarray(w_last.T, dtype=np.float32)

    if "nc" not in _NC_CACHE:
        _NC_CACHE["nc"] = _build()
    nc = _NC_CACHE["nc"]

    in_maps = [
        {"xb": xf[b], "wqk": wqk, "wgT": wgT, "wl": wl} for b in range(B)
    ]
    r = run_bass_kernel_spmd(nc, in_maps, list(range(B)))
    out = np.stack([r.results[b]["out"] for b in range(B)], axis=0)
    return out.reshape(B, C, H, W).astype(np.float32)


# revision 30
# speedup vs baseline: 1.0215x; 1.0215x over previous
"""Trainium2 Bass kernel for the non-local (self-attention over spatial
positions) block.

Per batch b (8 batches -> one per NeuronCore):
    xf    = x[b]                       [C=128, N=4096]
    theta = w_theta @ xf               [64, N]
    phi   = w_phi   @ xf               [64, N]
    g     = w_g     @ xf               [64, N]
    attn  = softmax(theta^T phi)       [N, N]   (softmax over keys m)
    y     = g @ attn^T                 [64, N]
    out   = w_last @ y + xf            [128, N]

Design (per core), final (~157 us vs 346 us baseline):
 - scoresT[m, q] orientation (phi tiles stationary) so exp(scoresT)
   feeds the y matmul directly as the moving operand.
 - fp16 theta/phi: the PE moving-operand path is ~256 B/cycle, so the
   two concurrent 64-row score matmuls (m-tile pair in disjoint PE row
   groups) stream 2 cols/cycle -- 2x over f32r at any clock, and
   fp16's 10-bit mantissa keeps logit error ~10x below bf16's.
   (bf16 operands everywhere failed the 2e-2 gate; f32 dram bits fed
   straight into f32r matmuls compute garbage on hardware.)
 - Keeping total PE load low and never stalling it keeps the tensor
   clock at 2.4 GHz instead of the 1.2 GHz it drops to under stalls.
 - exp (16.8M elements, the #2 load) split 20/12 between ACT (real
   Exp) and DVE (Schraudolph bit-trick exp: one tensor_scalar affine
   with int16 output bitcast as bf16), interleaved evenly so neither
   engine builds a backlog. y/w_last matmuls are uniform bf16 (range
   needs bf16: unnormalized y and row sums reach e^44).
 - Row sums via a ones column in gt. The per-chunk epilogue is staged
   across the next chunk's pair stream; the rowsum reciprocal runs as
   [128, 4] via an SBUF->SBUF DMA scatter/gather (a [1, 512] DVE
   reciprocal costs 3.3 us and starves the exp stream).
 - Input DMA / projections / first q-chunk fused so the PE starts
   while the input streams in; weights DMA'd before the bulk input.
"""

import sys

import numpy as np

for _p in ("/opt/trn_rl_repo",):
    if _p not in sys.path:
        sys.path.insert(0, _p)

import concourse.bass as bass
from concourse import bacc
import concourse.mybir as mybir
import concourse.tile as tile
from concourse.alu_op_type import AluOpType
from concourse.bass_utils import run_bass_kernel_spmd

F32 = mybir.dt.float32
F16 = mybir.dt.float16
BF16 = mybir.dt.bfloat16
I16 = mybir.dt.int16

P = 128     # channels C / partition dim
CB = 64     # bottleneck channels
NQ = 4096   # spatial positions (64*64)
NMT = 32    # m (key) tiles of 128
NPAIR = 16  # m-tile pairs per q chunk
NQC = 8     # q chunks of 512

# Schraudolph exp in bf16: exp(s) ~= bitcast<bf16>(int16(A*s + B))
EXP_A = float(2**7 / np.log(2.0))
EXP_B = float(127.0 * 2**7 - 6.0)
# exp engine split: DVE takes the second tile of pairs not divisible
# by 4 (12 of 32 tiles, evenly interleaved); ACT takes the other 20.
# 16/16 measured slower (DVE becomes co-pacer with its epilogue work).
def _exp_on_dve(mi):
    return (mi % 2 == 1) and ((mi // 2) % 4 != 0)

_NC_CACHE = {}


def _build():
    nc = bacc.Bacc()
    x_in = nc.declare_dram_parameter("xb", [P, NQ], F32, isOutput=False)
    wqk_in = nc.declare_dram_parameter("wqk", [P, P], F32, isOutput=False)
    wg_in = nc.declare_dram_parameter("wgT", [P, CB], F32, isOutput=False)
    wl_in = nc.declare_dram_parameter("wl", [CB, P], F32, isOutput=False)
    out_d = nc.declare_dram_parameter("out", [P, NQ], F32, isOutput=True)

    with tile.TileContext(nc) as tc:
        with (
            tc.tile_pool(name="const", bufs=1) as const,
            tc.tile_pool(name="big", bufs=1) as big,
            tc.tile_pool(name="work", bufs=2) as work,
            tc.tile_pool(name="probs", bufs=9) as probs,
            tc.tile_pool(name="spool", bufs=7, space="PSUM") as spool,
            tc.tile_pool(name="ypool", bufs=1, space="PSUM") as ypool,
        ):
            # ---- small weights first (needed before any compute) ----
            wqk_f = const.tile([P, P], F32)
            wg_f = const.tile([P, CB], F32)
            wl_f = const.tile([CB, P], F32)
            nc.sync.dma_start(out=wqk_f, in_=wqk_in[:, :])
            nc.sync.dma_start(out=wg_f, in_=wg_in[:, :])
            nc.sync.dma_start(out=wl_f, in_=wl_in[:, :])
            wqk = const.tile([P, P], F16)
            wg = const.tile([P, CB], F16)
            wl = const.tile([CB, P], BF16)
            nc.vector.tensor_copy(wqk, wqk_f)
            nc.vector.tensor_copy(wg, wg_f)
            nc.vector.tensor_copy(wl, wl_f)

            xb = big.tile([P, NQ], F32)
            xb16 = big.tile([P, NQ], F16)
            # theta/phi duplicated into both partition halves so score
            # matmuls for two m-tiles run concurrently in PE row groups
            theta = big.tile([P, NQ], F16)
            phi = big.tile([P, NQ], F16)
            # gT in 65-col slots; col 64 = ones for the row-sum trick
            gt = big.tile([P, NMT * (CB + 1)], BF16)
            nc.vector.memset(gt, 1.0)
            gt3 = gt.rearrange("p (m c) -> p m c", c=CB + 1)

            # ---------------- pipelined helpers ----------------
            qof = [qc * 512 for qc in range(NQC)]

            def score_pair(qc, pi):
                """Two concurrent 64-row score matmuls for m-tiles
                2*pi (rows 0:64) and 2*pi+1 (rows 64:128)."""
                q = qof[qc]
                sa = spool.tile([P, 512], F32, tag="s")
                nc.tensor.matmul(
                    sa, phi[0:CB, (2 * pi) * 128:(2 * pi + 1) * 128],
                    theta[0:CB, q:q + 512], start=True, stop=True,
                )
                sb = spool.tile([P, 512], F32, tag="s")
                nc.tensor.matmul(
                    sb, phi[CB:P, (2 * pi + 1) * 128:(2 * pi + 2) * 128],
                    theta[CB:P, q:q + 512], start=True, stop=True,
                )
                return sa, sb

            def exp_tile(mi, sp, split=False):
                pb = probs.tile([P, 512], BF16, tag="pb")
                if split:
                    # tail of the last chunk: halve drain latency by
                    # computing one half on each engine concurrently
                    nc.scalar.activation(
                        pb[:, 0:256], sp[:, 0:256],
                        mybir.ActivationFunctionType.Exp,
                    )
                    nc.vector.tensor_scalar(
                        pb.bitcast(I16)[:, 256:512], sp[:, 256:512],
                        EXP_A, EXP_B, AluOpType.mult, AluOpType.add,
                    )
                elif _exp_on_dve(mi):
                    nc.vector.tensor_scalar(
                        pb.bitcast(I16), sp, EXP_A, EXP_B,
                        AluOpType.mult, AluOpType.add,
                    )
                else:
                    nc.scalar.activation(
                        pb, sp, mybir.ActivationFunctionType.Exp
                    )
                return pb

            def y_mm(yps, mi, pb):
                nc.tensor.matmul(
                    yps,
                    gt[:, mi * (CB + 1):(mi + 1) * (CB + 1)],
                    pb,
                    start=(mi == 0), stop=(mi == NMT - 1),
                )

            # per-chunk epilogue, staged across the next chunk's pair
            # stream so no single engine stalls the PE. The rowsum
            # reciprocal is done as [128, 4] via an SBUF->SBUF DMA
            # scatter/gather: a [1, 512] DVE reciprocal costs 3.3us
            # (free-size bound) and starves the DVE exp stream, while
            # [128, 4] costs ~0.25us.
            #   chunk end: yu + rowsum copies (DVE, frees yps)
            #   pi==2: DMA scatter rowsum -> [128, 4]
            #   pi==3: reciprocal [128, 4] (DVE)
            #   pi==4: DMA gather -> [1, 512]
            #   pi==5: partition broadcast (gpsimd)
            #   pi==6: w_last matmul (PE)
            #   pi==7: normalize multiply (DVE, straight out of PSUM)
            #   pi==8: residual add (DVE)
            #   pi==9: output DMA
            def epi_stage(st, stage):
                qc = st["qc"]
                if stage == 2:
                    st["rsq"] = work.tile([P, 4], F32, tag="rsq", name="rsq")
                    nc.sync.dma_start(out=st["rsq"], in_=st["rs"])
                elif stage == 3:
                    st["rqi"] = work.tile([P, 4], F32, tag="rqi", name="rqi")
                    nc.vector.reciprocal(st["rqi"], st["rsq"])
                elif stage == 4:
                    st["rinv"] = work.tile([1, 512], F32, tag="rinv", name="rinv")
                    nc.sync.dma_start(out=st["rinv"], in_=st["rqi"])
                elif stage == 5:
                    st["rb"] = work.tile([P, 512], F32, tag="rb", name="rb")
                    nc.gpsimd.partition_broadcast(st["rb"], st["rinv"])
                elif stage == 6:
                    st["op"] = spool.tile([P, 512], F32, tag="s", name="op")
                    nc.tensor.matmul(st["op"], wl, st["yu"],
                                     start=True, stop=True)
                elif stage == 7:
                    st["ob"] = work.tile([P, 512], F32, tag="ob", name="ob")
                    nc.vector.tensor_mul(st["ob"], st["op"], st["rb"])
                elif stage == 8:
                    st["ob2"] = work.tile([P, 512], F32, tag="ob2", name="ob2")
                    nc.vector.tensor_add(
                        st["ob2"], st["ob"], xb[:, qof[qc]:qof[qc] + 512]
                    )
                elif stage == 9:
                    nc.sync.dma_start(
                        out=out_d[:, qof[qc]:qof[qc] + 512], in_=st["ob2"]
                    )

            EPI_STAGES = range(2, 10)

            def epi_begin(qc, yps):
                # rs first: it heads the reciprocal DMA chain (longest
                # pole of the epilogue); nothing needs yu until pi==6
                rs = work.tile([1, 512], F32, tag="rs")
                nc.vector.tensor_copy(rs, yps[CB:CB + 1, :])
                yu = work.tile([CB, 512], BF16, tag="yu")
                nc.vector.tensor_copy(yu, yps[0:CB, :])  # frees yps
                return {"qc": qc, "yps": yps, "yu": yu, "rs": rs}

            def pair_tiles(qc, yps, pi, look=2):
                """Scores+exp for pair pi, y matmuls for pair pi-look."""
                sa, sb = score_pair(qc, pi)
                split = (qc == NQC - 1 and pi >= NPAIR - 3)
                pbq[2 * pi] = exp_tile(2 * pi, sa, split)
                pbq[2 * pi + 1] = exp_tile(2 * pi + 1, sb, split)
                pj = pi - look
                if pj >= 0:
                    for mi in (2 * pj, 2 * pj + 1):
                        y_mm(yps, mi, pbq.pop(mi))

            def drain_y(yps, look=2):
                for pj in range(NPAIR - look, NPAIR):
                    for mi in (2 * pj, 2 * pj + 1):
                        y_mm(yps, mi, pbq.pop(mi))

            # ---------------- init fused with q-chunk 0 ----------------
            # Per 512-col xb chunk j: DMA, theta/phi projection, 4 gT
            # projections; from j>=1 also run q-chunk-0 score/exp/y for
            # the m-tile pairs whose phi/gt landed in iteration j-1.
            yps0 = ypool.tile([CB + 1, 512], F32, tag="y")
            pbq = {}  # mi -> pb tile awaiting its y matmul

            for j in range(8):
                cs = slice(j * 512, (j + 1) * 512)
                nc.sync.dma_start(out=xb[:, cs], in_=x_in[:, cs])
                nc.scalar.copy(xb16[:, cs], xb[:, cs])
                ps = spool.tile([P, 512], F32, tag="s")
                nc.tensor.matmul(ps, wqk, xb16[:, cs], start=True, stop=True)
                # theta lower half is partition-aligned -> ACT engine;
                # the shifted copies (DVE only) fill the other halves
                nc.scalar.copy(theta[0:CB, cs], ps[0:CB, :])
                nc.vector.tensor_copy(phi[0:CB, cs], ps[CB:P, :])
                nc.vector.tensor_copy(phi[CB:P, cs], ps[CB:P, :])
                if j == 0:
                    nc.vector.tensor_copy(theta[CB:P, cs], ps[0:CB, :])
                gp = spool.tile([P, 512], F32, tag="s")
                gp3 = gp.rearrange("p (m c) -> p m c", c=CB)
                for k in range(4):
                    mi = 4 * j + k
                    nc.tensor.matmul(
                        gp3[:, k, :], xb16[:, mi * 128:(mi + 1) * 128], wg,
                        start=True, stop=True,
                    )
                nc.scalar.copy(gt3[:, 4 * j:4 * j + 4, 0:CB], gp3[:, 0:4, :])
                if j >= 1:
                    for pi in (2 * (j - 1), 2 * (j - 1) + 1):
                        pair_tiles(0, yps0, pi)
            for pi in (14, 15):
                pair_tiles(0, yps0, pi)
            drain_y(yps0)
            # upper-theta for chunk 1 (needed at its start)
            c1 = slice(512, 1024)
            nc.vector.tensor_copy(theta[CB:P, c1], theta[0:CB, c1])
            pending = epi_begin(0, yps0)

            # ---------------- q-chunks 1..7 ----------------
            for qc in range(1, NQC):
                yps = ypool.tile([CB + 1, 512], F32, tag="y")
                for pi in range(NPAIR):
                    pair_tiles(qc, yps, pi)
                    if pending is not None and 2 <= pi <= 9:
                        epi_stage(pending, pi)
                    if pi == 8 and qc < NQC - 1:
                        # upper-theta for the next chunk, off-peak on DVE
                        cn = slice(qof[qc + 1], qof[qc + 1] + 512)
                        nc.vector.tensor_copy(theta[CB:P, cn], theta[0:CB, cn])
                drain_y(yps)
                pending = epi_begin(qc, yps)

            # final epilogue: w_last matmul (stage 6) early -- it only
            # needs yu, and runs on the PE concurrently with the
            # reciprocal DMA round-trip
            for stage in (2, 6, 3, 4, 5, 7, 8, 9):
                epi_stage(pending, stage)

    nc.finalize()
    return nc


def kernel(x, w_theta, w_phi, w_g, w_last):
    B, C, H, W = x.shape
    N = H * W
    xf = np.ascontiguousarray(x.reshape(B, C, N), dtype=np.float32)
    wqk = np.ascontiguousarray(
        np.concatenate([w_theta.T, w_phi.T], axis=1), dtype=np.float32
    )
    wgT = np.ascontiguousarray(w_g.T, dtype=np.float32)
    wl = np.ascontiguousarray(w_last.T, dtype=np.float32)

    if "nc" not in _NC_CACHE:
        _NC_CACHE["nc"] = _build()
    nc = _NC_CACHE["nc"]

    in_maps = [
        {"xb": xf[b], "wqk": wqk, "wgT": wgT, "wl": wl} for b in range(B)
    ]
    r = run_bass_kernel_spmd(nc, in_maps, list(range(B)))
    out = np.stack([r.results[b]["out"] for b in range(B)], axis=0)
    return out.reshape(B, C, H, W).astype(np.float32)
